# revision 4
# baseline (speedup 1.0000x reference)
"""Causal self-attention (B=4, T=2048, C=1024, H=16, D=64) on 8 TRN2 NeuronCores.

Sharding: core c = 2*b + g handles batch b (0..3) and head-group g (0..1),
i.e. 8 heads per core (4 head-pairs). Column-parallel QKV, row-parallel
c_proj; host sums the two partial outputs per batch.

Per-core pipeline (chunk-pipelined A->B->C):
  A(n): QKV projection for 512-token chunk n.
        qT/kT produced as [head-pair dims (128), tokens] (f32r),
        v produced as [tokens, head, dim(+ones col)] (f32r).
  B(n): flash attention, scores transposed: attT[k, q] = kT.T-ish via
        PE row-packed head pairs (contraction d=64, two heads in
        partitions 0:64 / 64:128). exp on ACT (PSUM->SBUF, f32r out),
        AV with ones-augmented V (M=65) accumulating yT + denominator.
        Deferred normalization: reciprocal (DVE) + partition_broadcast
        (GpSimd) + multiply (DVE).
  C(n): row-parallel c_proj partial output for chunk n.

Matmuls run in float32r (TF32-like, full PE rate at free dim >= 256);
fp32 accumulation in PSUM. End-to-end rel err ~2e-4.

Biases: b_attn q/k parts added on-device (per-partition scalar add);
v-bias and b_proj folded into a host-side output correction
(softmax rows sum to 1 -> y gets +b_v exactly).
"""

import numpy as np

import concourse.bass as bass
from concourse import bacc, tile, mybir, bass_utils

P = 128
T = 2048
C = 1024
NH = 16          # total heads
D = 64
NCORES = 8
NCH = 4          # 512-token chunks
QC = 512
NKT = T // P     # 16 k tiles
f32 = mybir.dt.float32
f32r = mybir.dt.float32r
Exp = mybir.ActivationFunctionType.Exp
ADD = mybir.AluOpType.add
MUL = mybir.AluOpType.mult

_CACHE = {}


def _build():
    nc = bacc.Bacc("TRN2", target_bir_lowering=False, debug=False)
    xT = nc.dram_tensor("xT", [C, T], f32, kind="ExternalInput").ap()
    wqkT = nc.dram_tensor("wqkT", [C, 1024], f32, kind="ExternalInput").ap()
    wvT = nc.dram_tensor("wvT", [C, 512], f32, kind="ExternalInput").ap()
    wpT = nc.dram_tensor("wpT", [512, C], f32, kind="ExternalInput").ap()
    bqk = nc.dram_tensor("bqk", [1024, 1], f32, kind="ExternalInput").ap()
    tri = nc.dram_tensor("tri", [P, P], f32, kind="ExternalInput").ap()
    out = nc.dram_tensor("out", [T, C], f32, kind="ExternalOutput").ap()

    with tile.TileContext(nc) as tc:
        with tc.tile_pool(name="pers", bufs=1) as pers, \
             tc.tile_pool(name="wstage", bufs=1) as wstage, \
             tc.tile_pool(name="xf", bufs=2) as xfp, \
             tc.tile_pool(name="xr", bufs=1) as xrp, \
             tc.tile_pool(name="qpool", bufs=2) as qpool, \
             tc.tile_pool(name="epool", bufs=3) as epool, \
             tc.tile_pool(name="fin", bufs=1) as fin, \
             tc.tile_pool(name="ypool", bufs=2) as ypool, \
             tc.tile_pool(name="opool", bufs=2) as opool, \
             tc.tile_pool(name="a_ps", bufs=2, space="PSUM") as a_ps, \
             tc.tile_pool(name="qk_ps", bufs=1, space="PSUM") as qk_ps, \
             tc.tile_pool(name="yA_ps", bufs=1, space="PSUM") as yA_ps, \
             tc.tile_pool(name="yB_ps", bufs=1, space="PSUM") as yB_ps, \
             tc.tile_pool(name="c_ps", bufs=2, space="PSUM") as c_ps:

            # ---- persistent weights (DMA fp32, cast to f32r) ----
            wqk_sb, wv_sb, wp_sb = [], [], []
            for s in range(8):
                wf = wstage.tile([P, 1024], f32, tag="wstage")
                nc.sync.dma_start(wf[:], wqkT[s * P:(s + 1) * P, :])
                wr = pers.tile([P, 1024], f32r, tag=f"wqk{s}", name=f"wqk{s}")
                nc.vector.tensor_copy(wr[:], wf[:])
                wqk_sb.append(wr)
            for s in range(8):
                wf = wstage.tile([P, 512], f32, tag="wstage")
                nc.sync.dma_start(wf[:], wvT[s * P:(s + 1) * P, :])
                wr = pers.tile([P, 512], f32r, tag=f"wv{s}", name=f"wv{s}")
                nc.vector.tensor_copy(wr[:], wf[:])
                wv_sb.append(wr)
            for s in range(4):
                wf = wstage.tile([P, 1024], f32, tag="wstage")
                nc.sync.dma_start(wf[:], wpT[s * P:(s + 1) * P, :])
                wr = pers.tile([P, 1024], f32r, tag=f"wp{s}", name=f"wp{s}")
                nc.vector.tensor_copy(wr[:], wf[:])
                wp_sb.append(wr)
            bqk_sb = pers.tile([P, 8], f32)
            nc.sync.dma_start(bqk_sb[:], bqk.rearrange("(m p) o -> p (m o)", p=P))
            tri_sb = pers.tile([P, P], f32)
            nc.sync.dma_start(tri_sb[:], tri)
            ones_sb = pers.tile([P, 8], f32)
            nc.vector.memset(ones_sb[:], 1.0)

            # persistent activations
            k_sb = [pers.tile([P, T], f32r, tag=f"k{p}", name=f"k{p}") for p in range(4)]
            v_sb = [pers.tile([P, 8, 65], f32r, tag=f"v{t}", name=f"v{t}") for t in range(NKT)]
            q_tiles = {}   # (p, n) -> tile
            y_tiles = {}   # (p, n) -> tile

            def phase_a(n):
                xr = xrp.tile([P, 8, QC], f32r, tag="xr")
                for s in range(8):
                    xf = xfp.tile([P, QC], f32, tag="xf")
                    nc.sync.dma_start(xf[:], xT[s * P:(s + 1) * P, n * QC:(n + 1) * QC])
                    nc.vector.tensor_copy(xr[:, s, :], xf[:])
                # qT (m 0..3) and kT (m 4..7)
                for m in range(8):
                    ps = a_ps.tile([P, QC], f32, tag="aps")
                    for s in range(8):
                        nc.tensor.matmul(ps[:], wqk_sb[s][:, m * P:(m + 1) * P],
                                         xr[:, s, :], start=(s == 0), stop=(s == 7))
                    if m < 4:
                        qt = qpool.tile([P, QC], f32r, tag=f"q{m}")
                        nc.vector.tensor_scalar_add(qt[:], ps[:], bqk_sb[:, m:m + 1])
                        q_tiles[(m, n)] = qt
                    else:
                        nc.vector.tensor_scalar_add(k_sb[m - 4][:, n * QC:(n + 1) * QC],
                                                    ps[:], bqk_sb[:, m:m + 1])
                # v tiles for this chunk
                for ti in range(4):
                    t = 4 * n + ti
                    ps = a_ps.tile([P, 8, D], f32, tag="aps")
                    for s in range(8):
                        nc.tensor.matmul(ps[:], xr[:, s, ti * P:(ti + 1) * P],
                                         wv_sb[s][:], start=(s == 0), stop=(s == 7))
                    nc.vector.tensor_copy(v_sb[t][:, :, 64:65], ones_sb[:, :, None])
                    nc.vector.tensor_copy(v_sb[t][:, :, 0:64], ps[:])

            def phase_b(n):
                for p in range(4):
                    psy = [
                        yA_ps.tile([65, QC], f32, tag="psyA", name=f"psyA_{n}_{p}"),
                        yB_ps.tile([65, QC], f32, tag="psyB", name=f"psyB_{n}_{p}"),
                    ]
                    last = 4 * n + 3
                    qt = q_tiles[(p, n)]
                    for j in range(4 * n + 4):
                        diag = j >= 4 * n
                        o = P * (j - 4 * n) if diag else 0
                        ps_g = qk_ps.tile([P, 2, QC], f32, tag="qkg")
                        for h in range(2):
                            b0 = h * 64
                            nc.tensor.matmul(ps_g[:, h, o:], k_sb[p][b0:b0 + 64, j * P:(j + 1) * P],
                                             qt[b0:b0 + 64, o:], start=True, stop=True)
                        if diag:
                            nc.vector.tensor_tensor(
                                ps_g[:, :, o:o + P], ps_g[:, :, o:o + P],
                                tri_sb[:, None, :].to_broadcast((P, 2, P)), ADD)
                        e = epool.tile([P, 2, QC], f32r, tag="e")
                        nc.scalar.activation(e[:, :, o:], ps_g[:, :, o:], Exp)
                        for h in range(2):
                            nc.tensor.matmul(psy[h][:, o:], v_sb[j][:, 2 * p + h, :],
                                             e[:, h, o:], start=(j == 0), stop=(j == last))
                    yt = ypool.tile([P, QC], f32r, tag=f"y{p}")
                    for h in range(2):
                        r = fin.tile([1, QC], f32, tag="r")
                        nc.vector.reciprocal(r[:], psy[h][64:65, :])
                        rb = fin.tile([64, QC], f32, tag="rb")
                        nc.gpsimd.partition_broadcast(rb[:], r[:])
                        nc.vector.tensor_tensor(yt[h * 64:(h + 1) * 64, :],
                                                psy[h][0:64, :], rb[:], MUL)
                    y_tiles[(p, n)] = yt

            def phase_c(n):
                for ti in range(4):
                    t = 4 * n + ti
                    for cc in range(2):
                        ps = c_ps.tile([P, QC], f32, tag="cps")
                        for s in range(4):
                            nc.tensor.matmul(ps[:], y_tiles[(s, n)][:, ti * P:(ti + 1) * P],
                                             wp_sb[s][:, cc * QC:(cc + 1) * QC],
                                             start=(s == 0), stop=(s == 3))
                        ob = opool.tile([P, QC], f32, tag="ob")
                        nc.vector.tensor_copy(ob[:], ps[:])
                        nc.sync.dma_start(out[t * P:(t + 1) * P, cc * QC:(cc + 1) * QC], ob[:])

            # chunk-pipelined emission
            phase_a(0)
            phase_a(1)
            phase_b(0)
            phase_c(0)
            phase_a(2)
            phase_b(1)
            phase_c(1)
            phase_a(3)
            phase_b(2)
            phase_c(2)
            phase_b(3)
            phase_c(3)

    nc.compile()
    return nc


def _prep_core_inputs(c, x, w_attn, b_attn):
    b, g = divmod(c, 2)
    heads = [g * 8 + 2 * p + e for p in range(4) for e in range(2)]
    qrows = np.concatenate([np.arange(h * D, (h + 1) * D) for h in heads])
    # wqkT columns: q pairs (scaled 1/8) then k pairs
    wq = w_attn[qrows, :] * 0.125
    wk = w_attn[C + qrows, :]
    wqkT = np.ascontiguousarray(np.concatenate([wq, wk], 0).T)
    wvT = np.ascontiguousarray(w_attn[2 * C + qrows, :].T)
    bqk = np.concatenate([b_attn[qrows] * 0.125, b_attn[C + qrows]]).reshape(1024, 1)
    xTc = np.ascontiguousarray(x[b].T)
    return {
        "xT": xTc.astype(np.float32),
        "wqkT": wqkT.astype(np.float32),
        "wvT": wvT.astype(np.float32),
        "bqk": bqk.astype(np.float32),
    }


def _prep_proj(c, w_proj):
    g = c % 2
    heads = [g * 8 + 2 * p + e for p in range(4) for e in range(2)]
    ch = np.concatenate([np.arange(h * D, (h + 1) * D) for h in heads])
    return np.ascontiguousarray(w_proj[:, ch].T).astype(np.float32)


def _tri_mask():
    k = np.arange(P)[:, None]
    q = np.arange(P)[None, :]
    return np.where(q >= k, 0.0, -1e30).astype(np.float32)


def kernel(x, w_attn, b_attn, w_proj, b_proj):
    x = np.asarray(x, dtype=np.float32)
    w_attn = np.asarray(w_attn, dtype=np.float32)
    b_attn = np.asarray(b_attn, dtype=np.float32)
    w_proj = np.asarray(w_proj, dtype=np.float32)
    b_proj = np.asarray(b_proj, dtype=np.float32)

    if "nc" not in _CACHE:
        _CACHE["nc"] = _build()
    nc = _CACHE["nc"]

    tri = _tri_mask()
    in_maps = []
    for c in range(NCORES):
        m = _prep_core_inputs(c, x, w_attn, b_attn)
        m["wpT"] = _prep_proj(c, w_proj)
        m["tri"] = tri
        in_maps.append(m)

    res = bass_utils.run_bass_kernel_spmd(nc, in_maps, core_ids=list(range(NCORES)))
    outs = [r["out"] for r in res.results]

    B = x.shape[0]
    corr = (b_attn[2 * C:] @ w_proj.T + b_proj).astype(np.float32)
    full = np.empty((B, T, C), np.float32)
    for b in range(B):
        full[b] = outs[2 * b] + outs[2 * b + 1] + corr
    return full


# revision 23
# speedup vs baseline: 1.0542x; 1.0542x over previous
"""Causal self-attention (B=4, T=2048, C=1024, H=16, D=64) on 8 TRN2 NeuronCores.

Sharding: core c = 2*b + g handles batch b (0..3) and head-group g (0..1),
i.e. 8 heads per core (4 head-pairs). Column-parallel QKV, row-parallel
c_proj; host sums the two partial outputs per batch.

Per-core pipeline (chunk-pipelined A->B->C):
  A(n): QKV projection for 512-token chunk n.
        qT/kT produced as [head-pair dims (128), tokens] (f32r),
        v produced as [tokens, head, dim(+ones col)] (f32r).
  B(n): flash attention, scores transposed: attT[k, q] = kT.T-ish via
        PE row-packed head pairs (contraction d=64, two heads in
        partitions 0:64 / 64:128). exp on ACT (PSUM->SBUF, f32r out),
        AV with ones-augmented V (M=65) accumulating yT + denominator.
        Deferred normalization: reciprocal (DVE) + partition_broadcast
        (GpSimd) + multiply (DVE).
  C(n): row-parallel c_proj partial output for chunk n.

Matmuls run in float32r (TF32-like, full PE rate at free dim >= 256);
fp32 accumulation in PSUM. End-to-end rel err ~2e-4.

Biases: b_attn q/k parts added on-device (per-partition scalar add);
v-bias and b_proj folded into a host-side output correction
(softmax rows sum to 1 -> y gets +b_v exactly).
"""

import numpy as np

import concourse.bass as bass
from concourse import bacc, tile, mybir, bass_utils

P = 128
T = 2048
C = 1024
NH = 16          # total heads
D = 64
NCORES = 8
NCH = 4          # 512-token chunks
QC = 512
NKT = T // P     # 16 k tiles
f32 = mybir.dt.float32
f32r = mybir.dt.float32r
Exp = mybir.ActivationFunctionType.Exp
ADD = mybir.AluOpType.add
MUL = mybir.AluOpType.mult

_CACHE = {}


def _build():
    nc = bacc.Bacc("TRN2", target_bir_lowering=False, debug=False)
    xT = nc.dram_tensor("xT", [C, T], f32, kind="ExternalInput").ap()
    wqkT = nc.dram_tensor("wqkT", [C, 1024], f32, kind="ExternalInput").ap()
    wvT = nc.dram_tensor("wvT", [C, 512], f32, kind="ExternalInput").ap()
    wpT = nc.dram_tensor("wpT", [512, C], f32, kind="ExternalInput").ap()
    bqk = nc.dram_tensor("bqk", [1024, 1], f32, kind="ExternalInput").ap()
    tri = nc.dram_tensor("tri", [P, P], f32, kind="ExternalInput").ap()
    out = nc.dram_tensor("out", [T, C], f32, kind="ExternalOutput").ap()

    with tile.TileContext(nc) as tc:
        with tc.tile_pool(name="pers", bufs=1) as pers, \
             tc.tile_pool(name="wstage", bufs=3) as wstage, \
             tc.tile_pool(name="xf", bufs=2) as xfp, \
             tc.tile_pool(name="xr", bufs=1) as xrp, \
             tc.tile_pool(name="qpool", bufs=2) as qpool, \
             tc.tile_pool(name="epool", bufs=3) as epool, \
             tc.tile_pool(name="fin", bufs=1) as fin, \
             tc.tile_pool(name="ypool", bufs=2) as ypool, \
             tc.tile_pool(name="opool", bufs=1) as opool, \
             tc.tile_pool(name="a_ps", bufs=1, space="PSUM") as a_ps, \
             tc.tile_pool(name="qk_ps", bufs=2, space="PSUM") as qk_ps, \
             tc.tile_pool(name="yA_ps", bufs=1, space="PSUM") as yA_ps, \
             tc.tile_pool(name="yB_ps", bufs=1, space="PSUM") as yB_ps, \
             tc.tile_pool(name="c_ps", bufs=1, space="PSUM") as c_ps:

            # ---- persistent weights (DMA fp32, cast to f32r) ----
            wqk_sb, wv_sb, wp_sb = [], [], []
            for s in range(8):
                wr = pers.tile([P, 1024], f32r, tag=f"wqk{s}", name=f"wqk{s}")
                wqk_sb.append(wr)
            for s in range(8):
                wr = pers.tile([P, 512], f32r, tag=f"wv{s}", name=f"wv{s}")
                wv_sb.append(wr)
            for s in range(4):
                wr = pers.tile([P, 1024], f32r, tag=f"wp{s}", name=f"wp{s}")
                wp_sb.append(wr)

            def load_weights_qkv():
                for c2 in range(2):
                    for s in range(8):
                        wf = wstage.tile([P, QC], f32, tag="wstage")
                        nc.sync.dma_start(wf[:], wqkT[s * P:(s + 1) * P, c2 * QC:(c2 + 1) * QC])
                        nc.vector.tensor_copy(wqk_sb[s][:, c2 * QC:(c2 + 1) * QC], wf[:])
                        yield
                for s in range(8):
                    wf = wstage.tile([P, QC], f32, tag="wstage")
                    nc.sync.dma_start(wf[:], wvT[s * P:(s + 1) * P, :])
                    nc.vector.tensor_copy(wv_sb[s][:], wf[:])
                    yield

            def load_weights_proj():
                for s in range(4):
                    for c2 in range(2):
                        wf = wstage.tile([P, QC], f32, tag="wstage")
                        nc.sync.dma_start(wf[:], wpT[s * P:(s + 1) * P, c2 * QC:(c2 + 1) * QC])
                        nc.vector.tensor_copy(wp_sb[s][:, c2 * QC:(c2 + 1) * QC], wf[:])
                        yield

            bqk_sb = pers.tile([P, 8], f32)
            nc.sync.dma_start(bqk_sb[:], bqk.rearrange("(m p) o -> p (m o)", p=P))
            tri_sb = pers.tile([P, P], f32)
            nc.sync.dma_start(tri_sb[:], tri)
            ones_sb = pers.tile([P, 8], f32)
            nc.vector.memset(ones_sb[:], 1.0)
            # preload the exp table set during the prologue
            warm_sb = pers.tile([1, 1], f32)
            nc.scalar.activation(warm_sb[:], ones_sb[0:1, 0:1], Exp)

            # persistent activations
            k_sb = [pers.tile([P, T], f32r, tag=f"k{p}", name=f"k{p}") for p in range(4)]
            v_sb = [pers.tile([P, 8, 65], f32r, tag=f"v{t}", name=f"v{t}") for t in range(NKT)]
            q_tiles = {}   # (p, n) -> tile
            y_tiles = {}   # (p, n) -> tile

            def phase_a(n):
                xr = xrp.tile([P, 8, QC], f32r, tag="xr")
                for s in range(8):
                    xf = xfp.tile([P, QC], f32, tag="xf")
                    nc.gpsimd.dma_start(xf[:], xT[s * P:(s + 1) * P, n * QC:(n + 1) * QC])
                    nc.vector.tensor_copy(xr[:, s, :], xf[:])
                    if s % 4 == 3:
                        yield

                def emit_qk(m):
                    ps = a_ps.tile([P, QC], f32, tag="aps")
                    for s in range(8):
                        nc.tensor.matmul(ps[:], wqk_sb[s][:, m * P:(m + 1) * P],
                                         xr[:, s, :], start=(s == 0), stop=(s == 7))
                        if s == 3:
                            yield
                    if m < 4:
                        qt = qpool.tile([P, QC], f32r, tag=f"q{m}")
                        nc.vector.tensor_scalar_add(qt[:], ps[:], bqk_sb[:, m:m + 1])
                        q_tiles[(m, n)] = qt
                    else:
                        nc.vector.tensor_scalar_add(k_sb[m - 4][:, n * QC:(n + 1) * QC],
                                                    ps[:], bqk_sb[:, m:m + 1])
                    yield

                def emit_v(ti):
                    t = 4 * n + ti
                    ps = a_ps.tile([P, 8, D], f32, tag="aps")
                    for s in range(8):
                        nc.tensor.matmul(ps[:], xr[:, s, ti * P:(ti + 1) * P],
                                         wv_sb[s][:], start=(s == 0), stop=(s == 7))
                        if s == 3:
                            yield
                    nc.vector.tensor_copy(v_sb[t][:, :, 64:65], ones_sb[:, :, None])
                    nc.vector.tensor_copy(v_sb[t][:, :, 0:64], ps[:])
                    yield

                # pair-0 q/k first, then v tiles, then remaining pairs:
                # B(n) pair p unblocks as early as possible.
                yield from emit_qk(0)
                yield from emit_qk(4)
                for ti in range(4):
                    yield from emit_v(ti)
                for p in range(1, 4):
                    yield from emit_qk(p)
                    yield from emit_qk(4 + p)

            def phase_b(n):
                for p in range(4):
                    psy = [
                        yA_ps.tile([65, QC], f32, tag="psyA", name=f"psyA_{n}_{p}"),
                        yB_ps.tile([65, QC], f32, tag="psyB", name=f"psyB_{n}_{p}"),
                    ]
                    last = 4 * n + 3
                    qt = q_tiles[(p, n)]
                    for j in range(4 * n + 4):
                        diag = j >= 4 * n
                        o = P * (j - 4 * n) if diag else 0
                        ps_g = qk_ps.tile([P, 2, QC], f32, tag="qkg")
                        for h in range(2):
                            b0 = h * 64
                            nc.tensor.matmul(ps_g[:, h, o:], k_sb[p][b0:b0 + 64, j * P:(j + 1) * P],
                                             qt[b0:b0 + 64, o:], start=True, stop=True)
                        if diag:
                            nc.vector.tensor_tensor(
                                ps_g[:, :, o:o + P], ps_g[:, :, o:o + P],
                                tri_sb[:, None, :].to_broadcast((P, 2, P)), ADD)
                        e = epool.tile([P, 2, QC], f32r, tag="e")
                        nc.scalar.activation(e[:, :, o:], ps_g[:, :, o:], Exp)
                        for h in range(2):
                            nc.tensor.matmul(psy[h][:, o:], v_sb[j][:, 2 * p + h, :],
                                             e[:, h, o:], start=(j == 0), stop=(j == last))
                        yield
                    yt = ypool.tile([P, QC], f32r, tag=f"y{p}")
                    for h in range(2):
                        r = fin.tile([1, QC], f32, tag="r")
                        nc.vector.reciprocal(r[:], psy[h][64:65, :])
                        rb = fin.tile([64, QC], f32, tag="rb")
                        nc.gpsimd.partition_broadcast(rb[:], r[:])
                        nc.vector.tensor_tensor(yt[h * 64:(h + 1) * 64, :],
                                                psy[h][0:64, :], rb[:], MUL)
                    y_tiles[(p, n)] = yt
                    yield

            def phase_c(n):
                for ti in range(4):
                    t = 4 * n + ti
                    for cc in range(2):
                        ps = c_ps.tile([P, QC], f32, tag="cps")
                        for s in range(4):
                            nc.tensor.matmul(ps[:], y_tiles[(s, n)][:, ti * P:(ti + 1) * P],
                                             wp_sb[s][:, cc * QC:(cc + 1) * QC],
                                             start=(s == 0), stop=(s == 3))
                        ob = opool.tile([P, QC], f32, tag="ob")
                        nc.vector.tensor_copy(ob[:], ps[:])
                        nc.sync.dma_start(out[t * P:(t + 1) * P, cc * QC:(cc + 1) * QC], ob[:])
                        yield

            def chain(*gens):
                for g in gens:
                    yield from g

            def run_all(g):
                for _ in g:
                    pass

            _SENTINEL = object()

            def interleave_lazy(base, inject, rate):
                """Emit all of `base`; after each base step emit `rate` steps
                of `inject` (fractional rates accumulate). Leftover inject
                steps are emitted at the end."""
                inj_iter = iter(inject)
                acc = 0.0
                exhausted = False
                for _ in base:
                    if exhausted:
                        continue
                    acc += rate
                    while acc >= 1.0 and not exhausted:
                        acc -= 1.0
                        if next(inj_iter, _SENTINEL) is _SENTINEL:
                            exhausted = True
                for _ in inj_iter:
                    pass

            # ---- emission schedule ----
            # prologue: qkv weights, then A(0) up through pair-0 q/k and v
            run_all(load_weights_qkv())
            a0 = phase_a(0)
            for _ in range(8):
                next(a0)
            # B(n) yields: 4 * (4n+4 j-steps + 1 finalize)
            b_steps = [4 * (4 * n + 5) for n in range(4)]
            interleave_lazy(phase_b(0), chain(a0, load_weights_proj(), phase_a(1)),
                            (16 + 8 + 26) / b_steps[0])
            interleave_lazy(phase_b(1), chain(phase_a(2), phase_c(0)), 34 / b_steps[1])
            interleave_lazy(phase_b(2), chain(phase_a(3), phase_c(1)), 34 / b_steps[2])
            interleave_lazy(phase_b(3), phase_c(2), 8 / b_steps[3])
            run_all(phase_c(3))

    nc.compile()
    return nc


def _prep_core_inputs(c, x, w_attn, b_attn):
    b, g = divmod(c, 2)
    heads = [g * 8 + 2 * p + e for p in range(4) for e in range(2)]
    qrows = np.concatenate([np.arange(h * D, (h + 1) * D) for h in heads])
    # wqkT columns: q pairs (scaled 1/8) then k pairs
    wq = w_attn[qrows, :] * 0.125
    wk = w_attn[C + qrows, :]
    wqkT = np.ascontiguousarray(np.concatenate([wq, wk], 0).T)
    wvT = np.ascontiguousarray(w_attn[2 * C + qrows, :].T)
    bqk = np.concatenate([b_attn[qrows] * 0.125, b_attn[C + qrows]]).reshape(1024, 1)
    xTc = np.ascontiguousarray(x[b].T)
    return {
        "xT": xTc.astype(np.float32),
        "wqkT": wqkT.astype(np.float32),
        "wvT": wvT.astype(np.float32),
        "bqk": bqk.astype(np.float32),
    }


def _prep_proj(c, w_proj):
    g = c % 2
    heads = [g * 8 + 2 * p + e for p in range(4) for e in range(2)]
    ch = np.concatenate([np.arange(h * D, (h + 1) * D) for h in heads])
    return np.ascontiguousarray(w_proj[:, ch].T).astype(np.float32)


def _tri_mask():
    k = np.arange(P)[:, None]
    q = np.arange(P)[None, :]
    return np.where(q >= k, 0.0, -1e30).astype(np.float32)


def kernel(x, w_attn, b_attn, w_proj, b_proj):
    x = np.asarray(x, dtype=np.float32)
    w_attn = np.asarray(w_attn, dtype=np.float32)
    b_attn = np.asarray(b_attn, dtype=np.float32)
    w_proj = np.asarray(w_proj, dtype=np.float32)
    b_proj = np.asarray(b_proj, dtype=np.float32)

    if "nc" not in _CACHE:
        _CACHE["nc"] = _build()
    nc = _CACHE["nc"]

    tri = _tri_mask()
    in_maps = []
    for c in range(NCORES):
        m = _prep_core_inputs(c, x, w_attn, b_attn)
        m["wpT"] = _prep_proj(c, w_proj)
        m["tri"] = tri
        in_maps.append(m)

    res = bass_utils.run_bass_kernel_spmd(nc, in_maps, core_ids=list(range(NCORES)))
    outs = [r["out"] for r in res.results]

    B = x.shape[0]
    corr = (b_attn[2 * C:] @ w_proj.T + b_proj).astype(np.float32)
    full = np.empty((B, T, C), np.float32)
    for b in range(B):
        full[b] = outs[2 * b] + outs[2 * b + 1] + corr
    return full
